# revision 35
# baseline (speedup 1.0000x reference)
"""Trainium2 Bass kernel for nn_Dist_Loss (discriminative distance loss).

Strategy (8 NeuronCores, SPMD — one instruction stream, per-core data):
  - Host sorts each dataset's rows by class label. Symmetry: each row-tile
    belongs to a "suffix level" and computes only columns [B_level, 4096)
    of the sorted order, covering each unordered pair once (diff-class
    entries weighted 2 host-side, wider-than-needed coverage discarded
    per-partition). Row-to-level assignment is per-core host data; levels
    fall back to wider suffixes when a class prefix overflows capacity.
  - d2[i,j] = ||xi||^2 + ||xj||^2 - 2 xi.xj is produced by ONE K=66 fp16
    matmul per tile: lhsT = [-2*x^T; 1; 1], rhs = [x^T; y2_hi; y2_lo],
    with the ||xi||^2 + eps term folded into the ScalarE activation bias.
  - ScalarE computes D = sqrt(d2 + eps) straight out of PSUM, with
    accum_out giving per-partition (= per-row) sums over each
    class-pure column range (class boundaries of the sorted columns are
    compile-time constants, identical across cores).
  - Class centers: per-tile PE matmul onehot^T @ [x | 1] accumulated on
    SBUF.
  - Host buckets the per-row/per-class partial sums into compactness /
    separability using each core's row labels, corrects the diagonal
    (which reads sqrt(eps)), and finishes the tiny [4,4] center term.
"""

import os
import sys
from contextlib import ExitStack

import numpy as np

if "/opt/trn_rl_repo" not in sys.path:
    sys.path.insert(0, "/opt/trn_rl_repo")

import concourse.bacc as bacc
import concourse.bass as bass
import concourse.mybir as mybir
import concourse.tile as tile
from concourse.bass_utils import run_bass_kernel_spmd

N = 4096
DDIM = 64
K = DDIM + 2          # 64 features + y2_hi + y2_lo
CLASSES = 4
NCORES = 8
ROWS = N // NCORES    # 512 rows per core
TILE_P = 128
NTILES = ROWS // TILE_P  # 4 row-tiles per core per dataset
PSUM_W = 1536         # d2 PSUM window width (3 banks)
MM_W = 512            # matmul chunk width (1 PSUM bank)
EPS = 1e-3
ACC_COLS = 64         # accum output width (>= total pieces)

F16 = mybir.dt.float16
F32 = mybir.dt.float32

_program_cache: dict = {}


def _suffix_levels(bounds):
    """Symmetric-coverage levels: level i's row-tile computes columns
    [B_i, N) only. B_i = b_{k(i)} with k(i) = max k s.t. prefix_k <= 1024*i
    (capacity: each level holds 1024 rows = 8 cores x 128, and a row of
    class c may only ride a level with B_i <= b_c)."""
    Bs = []
    for i in range(CLASSES):
        k = 0
        for kk in range(CLASSES):
            if bounds[kk] <= 1024 * i:
                k = kk
        Bs.append(int(bounds[k]))
    return Bs


def _make_pieces_for_level(bounds, B):
    """Class-pure column pieces covering [B, N), PSUM_W-capped."""
    pieces = []
    for c in range(CLASSES):
        lo, hi = max(int(bounds[c]), B), int(bounds[c + 1])
        w0 = lo
        while w0 < hi:
            w1 = min(w0 + PSUM_W, hi)
            pieces.append((w0, w0, w1, c))
            w0 = w1
    return pieces


def _build_program(bounds_pair, reps=1):
    """Build the SPMD Bass program for the given per-dataset class
    boundaries (identical across cores). Returns (nc, layouts) where
    layouts[ds] = list of (w0, a, b, cls, acc_col) per row-tile piece.

    reps>1 repeats the whole compute body (identical work, overwriting
    the same accumulators) — used only for device-time measurement."""
    # acc column assignment: ds-major, then level (= row-tile), then piece
    layouts = []
    col = 0
    for ds in range(2):
        Bs = _suffix_levels(bounds_pair[ds])
        lay = []
        for t in range(NTILES):
            for (w0, a, b, cls) in _make_pieces_for_level(bounds_pair[ds], Bs[t]):
                lay.append((t, w0, a, b, cls, col))
                col += 1
        layouts.append(lay)
    ncols = col
    assert ncols <= ACC_COLS, f"too many accum pieces: {ncols}"

    nc = bacc.Bacc("TRN2", target_bir_lowering=False, debug=False,
                   num_devices=NCORES)

    rhs_d = [nc.dram_tensor(f"rhs{d}", [K, N], F16, kind="ExternalInput").ap()
             for d in range(2)]
    lhs_d = [nc.dram_tensor(f"lhs{d}", [K, ROWS], F16, kind="ExternalInput").ap()
             for d in range(2)]
    bias_d = [nc.dram_tensor(f"bias{d}", [NTILES, TILE_P], F32,
                             kind="ExternalInput").ap() for d in range(2)]
    xo_d = [nc.dram_tensor(f"xo{d}", [NTILES, TILE_P, DDIM + 1 + CLASSES], F32,
                           kind="ExternalInput").ap() for d in range(2)]
    acc_out = nc.dram_tensor("acc", [TILE_P, ACC_COLS], F32,
                             kind="ExternalOutput").ap()
    cent_out = nc.dram_tensor("cent", [CLASSES, 2 * (DDIM + 1)], F32,
                              kind="ExternalOutput").ap()

    with tile.TileContext(nc) as tc, ExitStack() as ctx:
        rhs_pool = ctx.enter_context(tc.tile_pool(name="rhs", bufs=2))
        lhs_pool = ctx.enter_context(tc.tile_pool(name="lhs", bufs=2))
        bias_pool = ctx.enter_context(tc.tile_pool(name="bias", bufs=2))
        xo_pool = ctx.enter_context(tc.tile_pool(name="xo", bufs=2))
        dscr_pool = ctx.enter_context(tc.tile_pool(name="dscr", bufs=4))
        accs_pool = ctx.enter_context(tc.tile_pool(name="accs", bufs=1))
        cacc_pool = ctx.enter_context(tc.tile_pool(name="cacc", bufs=1))
        d2_pool = ctx.enter_context(tc.tile_pool(name="d2", bufs=2, space="PSUM"))
        cent_pool = ctx.enter_context(tc.tile_pool(name="centp", bufs=2, space="PSUM"))

        acc_sb = accs_pool.tile([TILE_P, ACC_COLS], F32)
        cent_acc = cacc_pool.tile([CLASSES, 2 * (DDIM + 1)], F32)

        for _rep in range(reps):
            _build_body(nc, tc, layouts, rhs_d, lhs_d, bias_d, xo_d,
                        rhs_pool, lhs_pool, bias_pool, xo_pool,
                        dscr_pool, d2_pool, cent_pool, acc_sb, cent_acc,
                        acc_out=acc_out if _rep == reps - 1 else None)

        nc.scalar.dma_start(cent_out[:], cent_acc[:])

    nc.compile()
    return nc, layouts


DVE_MODE = ["all"]  # "none" | "half" | "all"


def _route_dve(ds, t, pidx):
    """Which pieces' accumulation runs on DVE (reduce) vs ACT (accum_out)."""
    m = DVE_MODE[0]
    if ds == 1 and t == NTILES - 1:
        return False  # keep the last tile on ACT accum: shorter kernel tail
    if m == "none":
        return False
    if m == "all":
        return True
    if m == "quarter":
        return pidx == 0
    if m == "threequarter":
        return pidx != 1
    return pidx % 2 == 0


def _build_body(nc, tc, layouts, rhs_d, lhs_d, bias_d, xo_d,
                rhs_pool, lhs_pool, bias_pool, xo_pool,
                dscr_pool, d2_pool, cent_pool, acc_sb, cent_acc,
                acc_out=None):
    XO_W = DDIM + 1 + CLASSES  # 69
    # Sqrt table-load warmup: tiny activation on a self-made tile so the
    # ~2.7us ACT table load overlaps the initial DMAs.
    warm = bias_pool.tile([TILE_P, 2], F32, tag="warm")
    nc.gpsimd.memset(warm[:, 0:1], 1.0)
    nc.scalar.activation(warm[:, 1:2], warm[:, 0:1],
                         mybir.ActivationFunctionType.Sqrt,
                         bias=warm[:, 0:1], scale=1.0)

    # small inputs on the SWDGE (gpsimd) queue so they don't serialize
    # behind the big rhs loads on the SP HWDGE queue
    bias_sb = []
    lhs_sb = []
    xo_sb = []
    for ds in range(2):
        bs = bias_pool.tile([TILE_P, NTILES], F32)
        nc.gpsimd.dma_start(bs[:], bias_d[ds].rearrange("t p -> p t"))
        bias_sb.append(bs)
        ls = lhs_pool.tile([K, ROWS], F16)
        nc.gpsimd.dma_start(ls[:], lhs_d[ds][:])
        lhs_sb.append(ls)
        xs = xo_pool.tile([TILE_P, NTILES * XO_W], F32)
        nc.gpsimd.dma_start(
            xs[:].rearrange("p (t c) -> p t c", t=NTILES),
            xo_d[ds].rearrange("t p c -> p t c"))
        xo_sb.append(xs)

    for ds in range(2):
        rhs_sb = rhs_pool.tile([K, N], F16)
        cuts = [0, 512, 1024, 2048, 3072, 4096] if ds == 0 else \
               [0, 1024, 2048, 3072, 4096]
        for j0, j1 in zip(cuts[:-1], cuts[1:]):
            nc.sync.dma_start(rhs_sb[:, j0:j1], rhs_d[ds][:, j0:j1])

        piece_by_tile: dict = {}
        for (t, w0, a, b, cls, col_) in layouts[ds]:
            piece_by_tile.setdefault(t, []).append((a, b, cls, col_))

        for t in range(NTILES):
            lhsT = lhs_sb[ds][:, t * TILE_P:(t + 1) * TILE_P]
            for pidx, (a, b, cls, col_) in enumerate(piece_by_tile[t]):
                d2 = d2_pool.tile([TILE_P, PSUM_W], F32)
                for c0 in range(a, b, MM_W):
                    c1 = min(c0 + MM_W, b)
                    nc.tensor.matmul(d2[:, c0 - a:c1 - a], lhsT,
                                     rhs_sb[:, c0:c1],
                                     start=True, stop=True)
                dsc = dscr_pool.tile([TILE_P, PSUM_W], F16)
                on_dve = _route_dve(ds, t, pidx)
                if on_dve:
                    nc.scalar.activation(
                        dsc[:, 0:b - a], d2[:, 0:b - a],
                        mybir.ActivationFunctionType.Sqrt,
                        bias=bias_sb[ds][:, t:t + 1], scale=1.0)
                    dsc2 = dscr_pool.tile([TILE_P, PSUM_W], F16, tag="dsc2")
                    nc.vector.tensor_scalar(
                        dsc2[:, 0:b - a], dsc[:, 0:b - a], 1.0, None,
                        op0=mybir.AluOpType.mult, op1=mybir.AluOpType.add,
                        accum_out=acc_sb[:, col_:col_ + 1])
                else:
                    nc.scalar.activation(
                        dsc[:, 0:b - a], d2[:, 0:b - a],
                        mybir.ActivationFunctionType.Sqrt,
                        bias=bias_sb[ds][:, t:t + 1], scale=1.0,
                        accum_out=acc_sb[:, col_:col_ + 1])

            # class centers: onehot^T @ [x | 1]
            xna = xo_sb[ds][:, t * XO_W:t * XO_W + DDIM + 1]
            ohe = xo_sb[ds][:, t * XO_W + DDIM + 1:(t + 1) * XO_W]
            cps = cent_pool.tile([CLASSES, DDIM + 1], F32)
            nc.tensor.matmul(cps[:], ohe, xna, start=True, stop=True)
            w = DDIM + 1
            if t == 0:
                nc.vector.tensor_copy(cent_acc[:, ds * w:(ds + 1) * w], cps[:])
            else:
                nc.vector.tensor_add(
                    cent_acc[:, ds * w:(ds + 1) * w],
                    cent_acc[:, ds * w:(ds + 1) * w], cps[:])

        if acc_out is not None:
            # drain accum columns as they complete (finer for ds1 tail)
            if ds == 0:
                cols = [col_ for (t, w0, a, b, cls, col_) in layouts[ds]]
                c0, c1 = min(cols), max(cols) + 1
                nc.sync.dma_start(acc_out[:, c0:c1], acc_sb[:, c0:c1])
            else:
                by_t: dict = {}
                for (t, w0, a, b, cls, col_) in layouts[ds]:
                    by_t.setdefault(t, []).append(col_)
                for t, cols in by_t.items():
                    c0, c1 = min(cols), max(cols) + 1
                    nc.sync.dma_start(acc_out[:, c0:c1], acc_sb[:, c0:c1])


def _prepare_dataset(data, labels):
    """Sort by class; build device arrays. Returns a dict of host arrays."""
    data = np.ascontiguousarray(np.asarray(data, dtype=np.float32))
    labels = np.asarray(labels).astype(np.int64)
    order = np.argsort(labels, kind="stable")
    xs = data[order]                       # [N, D] f32 sorted
    ls = labels[order]                     # [N]
    counts = np.bincount(ls, minlength=CLASSES)
    bounds = tuple(int(v) for v in np.concatenate([[0], np.cumsum(counts)]))
    xh = xs.astype(np.float16)
    x2 = (xh.astype(np.float64) ** 2).sum(1)
    y2_hi = x2.astype(np.float16)
    y2_lo = (x2 - y2_hi.astype(np.float64)).astype(np.float16)
    rhs = np.ascontiguousarray(
        np.concatenate([xh.T, y2_hi[None], y2_lo[None]], axis=0))  # [66, N] f16
    lhsT = np.ascontiguousarray(np.concatenate(
        [(-2.0 * xh.astype(np.float32)).astype(np.float16).T,
         np.ones((2, N), np.float16)], axis=0))                    # [66, N] f16
    bias = (x2 + EPS).astype(np.float32).reshape(N, 1)             # [N,1] f32
    xo = np.ascontiguousarray(np.concatenate(
        [xs, np.ones((N, 1), np.float32),
         np.eye(CLASSES, dtype=np.float32)[ls]], axis=1))          # [N,69] f32
    # symmetric suffix-level row assignment: level t (cols [B_t, N)) takes
    # 1024 rows with b_class >= B_t; fill narrowest level first with the
    # highest classes (they fit the fewest levels... actually widest-eligible)
    Bs = _suffix_levels(bounds)
    unassigned = list(range(N - 1, -1, -1))  # highest class first (sorted order)
    level_rows = []
    for t in range(CLASSES - 1, -1, -1):
        take = []
        rest = []
        for r in unassigned:
            if len(take) < 1024 and int(bounds[ls[r]]) >= Bs[t]:
                take.append(r)
            else:
                rest.append(r)
        assert len(take) == 1024, f"level {t} underfilled: {len(take)}"
        take.sort()
        level_rows.append(take)
        unassigned = rest
    level_rows = level_rows[::-1]  # index by level t
    return dict(xs=xs, ls=ls, bounds=bounds, rhs=rhs, lhsT=lhsT,
                bias=bias, xo=xo, level_rows=np.array(level_rows))


def _run(inputs, trace=False):
    """Core implementation. Returns (result_scalar, BassKernelResults)."""
    assert np.asarray(inputs["source_data"]).shape == (N, DDIM), \
        "kernel compiled for source/target [4096, 64]"
    preps = [
        _prepare_dataset(inputs["source_data"], inputs["source_labels"]),
        _prepare_dataset(inputs["target_data"], inputs["target_labels"]),
    ]
    alpha = float(np.asarray(inputs["alpha"], dtype=np.float64))
    bounds_pair = (preps[0]["bounds"], preps[1]["bounds"])

    if bounds_pair not in _program_cache:
        _program_cache[bounds_pair] = _build_program(bounds_pair)
    nc, layouts = _program_cache[bounds_pair]

    in_maps = []
    perms = []  # [core][ds] -> row permutation (level-major, 512 rows)
    for core in range(NCORES):
        m = {}
        cp = []
        for ds, p in enumerate(preps):
            perm = np.concatenate(
                [p["level_rows"][t][core * TILE_P:(core + 1) * TILE_P]
                 for t in range(NTILES)])
            cp.append(perm)
            m[f"rhs{ds}"] = p["rhs"]
            m[f"lhs{ds}"] = np.ascontiguousarray(p["lhsT"][:, perm])
            m[f"bias{ds}"] = np.ascontiguousarray(
                p["bias"][perm].reshape(NTILES, TILE_P))
            m[f"xo{ds}"] = np.ascontiguousarray(
                p["xo"][perm].reshape(NTILES, TILE_P, -1))
        perms.append(cp)
        in_maps.append(m)

    res = run_bass_kernel_spmd(nc, in_maps, core_ids=list(range(NCORES)),
                               trace=False)

    # ---- host combination (symmetric coverage) ----
    # each computed (row c, col-class b) entry: b == c -> same-class pair,
    # counted once (incl. diagonal); b > c -> diff-class pair, represents
    # both orderings (weight 2); b < c -> duplicate coverage, discarded.
    diag = N * np.sqrt(EPS)
    total = 0.0
    cent_sum = np.zeros((CLASSES, 2 * (DDIM + 1)), np.float64)
    terms = []
    for ds, p in enumerate(preps):
        C = 0.0
        S2 = 0.0
        ls = p["ls"]
        for core in range(NCORES):
            acc = res.results[core]["acc"].astype(np.float64)  # [128, ACC_COLS]
            for (t, w0, a, b, cls, col_) in layouts[ds]:
                vec = acc[:, col_]
                rowcls = ls[perms[core][ds][t * TILE_P:(t + 1) * TILE_P]]
                C += vec[rowcls == cls].sum()
                S2 += vec[rowcls < cls].sum()
        C -= diag
        S = 2.0 * S2
        terms.append((0.5 * C - alpha * S) / N)
        total += terms[-1]
    for core in range(NCORES):
        cent_sum += res.results[core]["cent"].astype(np.float64)
    w = DDIM + 1
    cs = cent_sum[:, :w]
    ct = cent_sum[:, w:]
    src_centers = cs[:, :DDIM] / cs[:, DDIM:DDIM + 1]
    tgt_centers = ct[:, :DDIM] / ct[:, DDIM:DDIM + 1]
    diff = src_centers - tgt_centers
    total += float(np.sqrt((diff ** 2).sum(1)).mean())

    return np.array(total, dtype=np.float32), res


def kernel(**inputs):
    out, _ = _run(inputs, trace=False)
    return out


def run_traced(inputs):
    """For test harness: returns (output, BassKernelResults). NTFF tracing
    is unavailable on this axon client, so this is the untraced run."""
    return _run(inputs, trace=False)


# revision 38
# speedup vs baseline: 1.0022x; 1.0022x over previous
"""Trainium2 Bass kernel for nn_Dist_Loss (discriminative distance loss).

Strategy (8 NeuronCores, SPMD — one instruction stream, per-core data):
  - Host sorts each dataset's rows by class label. Symmetry: each row-tile
    belongs to a "suffix level" and computes only columns [B_level, 4096)
    of the sorted order, covering each unordered pair once (diff-class
    entries weighted 2 host-side, wider-than-needed coverage discarded
    per-partition). Row-to-level assignment is per-core host data; levels
    fall back to wider suffixes when a class prefix overflows capacity.
  - d2[i,j] = ||xi||^2 + ||xj||^2 - 2 xi.xj is produced by ONE K=66 fp16
    matmul per tile: lhsT = [-2*x^T; 1; 1], rhs = [x^T; y2_hi; y2_lo],
    with the ||xi||^2 + eps term folded into the ScalarE activation bias.
  - ScalarE computes D = sqrt(d2 + eps) straight out of PSUM, with
    accum_out giving per-partition (= per-row) sums over each
    class-pure column range (class boundaries of the sorted columns are
    compile-time constants, identical across cores).
  - Class centers: per-tile PE matmul onehot^T @ [x | 1] accumulated on
    SBUF.
  - Host buckets the per-row/per-class partial sums into compactness /
    separability using each core's row labels, corrects the diagonal
    (which reads sqrt(eps)), and finishes the tiny [4,4] center term.
"""

import os
import sys
from contextlib import ExitStack

import numpy as np

if "/opt/trn_rl_repo" not in sys.path:
    sys.path.insert(0, "/opt/trn_rl_repo")

import concourse.bacc as bacc
import concourse.bass as bass
import concourse.mybir as mybir
import concourse.tile as tile
from concourse.bass_utils import run_bass_kernel_spmd

N = 4096
DDIM = 64
K = DDIM + 2          # 64 features + y2_hi + y2_lo
CLASSES = 4
NCORES = 8
ROWS = N // NCORES    # 512 rows per core
TILE_P = 128
NTILES = ROWS // TILE_P  # 4 row-tiles per core per dataset
PSUM_W = 1536         # d2 PSUM window width (3 banks)
MM_W = 512            # matmul chunk width (1 PSUM bank)
EPS = 1e-3
ACC_COLS = 64         # accum output width (>= total pieces)

F16 = mybir.dt.float16
F32 = mybir.dt.float32

_program_cache: dict = {}


def _suffix_levels(bounds):
    """Symmetric-coverage levels: level i's row-tile computes columns
    [B_i, N) only. B_i = b_{k(i)} with k(i) = max k s.t. prefix_k <= 1024*i
    (capacity: each level holds 1024 rows = 8 cores x 128, and a row of
    class c may only ride a level with B_i <= b_c)."""
    Bs = []
    for i in range(CLASSES):
        k = 0
        for kk in range(CLASSES):
            if bounds[kk] <= 1024 * i:
                k = kk
        Bs.append(int(bounds[k]))
    return Bs


def _make_pieces_for_level(bounds, B):
    """Class-pure column pieces covering [B, N), PSUM_W-capped."""
    pieces = []
    for c in range(CLASSES):
        lo, hi = max(int(bounds[c]), B), int(bounds[c + 1])
        w0 = lo
        while w0 < hi:
            w1 = min(w0 + PSUM_W, hi)
            pieces.append((w0, w0, w1, c))
            w0 = w1
    return pieces


def _build_program(bounds_pair, reps=1):
    """Build the SPMD Bass program for the given per-dataset class
    boundaries (identical across cores). Returns (nc, layouts) where
    layouts[ds] = list of (w0, a, b, cls, acc_col) per row-tile piece.

    reps>1 repeats the whole compute body (identical work, overwriting
    the same accumulators) — used only for device-time measurement."""
    # acc column assignment: ds-major, then level (= row-tile), then piece
    layouts = []
    col = 0
    for ds in range(2):
        Bs = _suffix_levels(bounds_pair[ds])
        lay = []
        for t in range(NTILES):
            for (w0, a, b, cls) in _make_pieces_for_level(bounds_pair[ds], Bs[t]):
                lay.append((t, w0, a, b, cls, col))
                col += 1
        layouts.append(lay)
    ncols = col
    assert ncols <= ACC_COLS, f"too many accum pieces: {ncols}"

    nc = bacc.Bacc("TRN2", target_bir_lowering=False, debug=False,
                   num_devices=NCORES)

    rhs_d = [nc.dram_tensor(f"rhs{d}", [K, N], F16, kind="ExternalInput").ap()
             for d in range(2)]
    lhs_d = [nc.dram_tensor(f"lhs{d}", [K, ROWS], F16, kind="ExternalInput").ap()
             for d in range(2)]
    bias_d = [nc.dram_tensor(f"bias{d}", [NTILES, TILE_P], F32,
                             kind="ExternalInput").ap() for d in range(2)]
    xo_d = [nc.dram_tensor(f"xo{d}", [NTILES, TILE_P, DDIM + 1 + CLASSES], F32,
                           kind="ExternalInput").ap() for d in range(2)]
    acc_out = nc.dram_tensor("acc", [TILE_P, ACC_COLS], F32,
                             kind="ExternalOutput").ap()
    cent_out = nc.dram_tensor("cent", [CLASSES, 2 * (DDIM + 1)], F32,
                              kind="ExternalOutput").ap()

    with tile.TileContext(nc) as tc, ExitStack() as ctx:
        rhs_pool = ctx.enter_context(tc.tile_pool(name="rhs", bufs=2))
        lhs_pool = ctx.enter_context(tc.tile_pool(name="lhs", bufs=2))
        bias_pool = ctx.enter_context(tc.tile_pool(name="bias", bufs=2))
        xo_pool = ctx.enter_context(tc.tile_pool(name="xo", bufs=2))
        dscr_pool = ctx.enter_context(tc.tile_pool(name="dscr", bufs=6))
        accs_pool = ctx.enter_context(tc.tile_pool(name="accs", bufs=1))
        cacc_pool = ctx.enter_context(tc.tile_pool(name="cacc", bufs=1))
        d2_pool = ctx.enter_context(tc.tile_pool(name="d2", bufs=2, space="PSUM"))
        cent_pool = ctx.enter_context(tc.tile_pool(name="centp", bufs=2, space="PSUM"))

        acc_sb = accs_pool.tile([TILE_P, ACC_COLS], F32)
        cent_acc = cacc_pool.tile([CLASSES, 2 * (DDIM + 1)], F32)

        for _rep in range(reps):
            _build_body(nc, tc, layouts, rhs_d, lhs_d, bias_d, xo_d,
                        rhs_pool, lhs_pool, bias_pool, xo_pool,
                        dscr_pool, d2_pool, cent_pool, acc_sb, cent_acc,
                        acc_out=acc_out if _rep == reps - 1 else None)

        nc.scalar.dma_start(cent_out[:], cent_acc[:])

    nc.compile()
    return nc, layouts


DVE_MODE = ["all"]  # "none" | "half" | "all"


def _route_dve(ds, t, pidx):
    """Which pieces' accumulation runs on DVE (reduce) vs ACT (accum_out)."""
    m = DVE_MODE[0]
    if ds == 1 and t == NTILES - 1:
        return False  # keep the last tile on ACT accum: shorter kernel tail
    if m == "none":
        return False
    if m == "all":
        return True
    if m == "quarter":
        return pidx == 0
    if m == "threequarter":
        return pidx != 1
    return pidx % 2 == 0


def _build_body(nc, tc, layouts, rhs_d, lhs_d, bias_d, xo_d,
                rhs_pool, lhs_pool, bias_pool, xo_pool,
                dscr_pool, d2_pool, cent_pool, acc_sb, cent_acc,
                acc_out=None):
    XO_W = DDIM + 1 + CLASSES  # 69
    # Sqrt table-load warmup: tiny activation on a self-made tile so the
    # ~2.7us ACT table load overlaps the initial DMAs.
    warm = bias_pool.tile([TILE_P, 2], F32, tag="warm")
    nc.gpsimd.memset(warm[:, 0:1], 1.0)
    nc.scalar.activation(warm[:, 1:2], warm[:, 0:1],
                         mybir.ActivationFunctionType.Sqrt,
                         bias=warm[:, 0:1], scale=1.0)

    # small inputs on the SWDGE (gpsimd) queue so they don't serialize
    # behind the big rhs loads on the SP HWDGE queue
    bias_sb = []
    lhs_sb = []
    xo_sb = []
    for ds in range(2):
        bs = bias_pool.tile([TILE_P, NTILES], F32)
        nc.gpsimd.dma_start(bs[:], bias_d[ds].rearrange("t p -> p t"))
        bias_sb.append(bs)
        ls = lhs_pool.tile([K, ROWS], F16)
        nc.gpsimd.dma_start(ls[:], lhs_d[ds][:])
        lhs_sb.append(ls)
        xs = xo_pool.tile([TILE_P, NTILES * XO_W], F32)
        nc.gpsimd.dma_start(
            xs[:].rearrange("p (t c) -> p t c", t=NTILES),
            xo_d[ds].rearrange("t p c -> p t c"))
        xo_sb.append(xs)

    for ds in range(2):
        rhs_sb = rhs_pool.tile([K, N], F16)
        cuts = [0, 512, 1024, 2048, 3072, 4096] if ds == 0 else \
               [0, 1024, 2048, 3072, 4096]
        for j0, j1 in zip(cuts[:-1], cuts[1:]):
            nc.sync.dma_start(rhs_sb[:, j0:j1], rhs_d[ds][:, j0:j1])

        piece_by_tile: dict = {}
        for (t, w0, a, b, cls, col_) in layouts[ds]:
            piece_by_tile.setdefault(t, []).append((a, b, cls, col_))

        for t in range(NTILES):
            lhsT = lhs_sb[ds][:, t * TILE_P:(t + 1) * TILE_P]
            for pidx, (a, b, cls, col_) in enumerate(piece_by_tile[t]):
                d2 = d2_pool.tile([TILE_P, PSUM_W], F32)
                for c0 in range(a, b, MM_W):
                    c1 = min(c0 + MM_W, b)
                    nc.tensor.matmul(d2[:, c0 - a:c1 - a], lhsT,
                                     rhs_sb[:, c0:c1],
                                     start=True, stop=True)
                dsc = dscr_pool.tile([TILE_P, PSUM_W], F16)
                on_dve = _route_dve(ds, t, pidx)
                if on_dve:
                    nc.scalar.activation(
                        dsc[:, 0:b - a], d2[:, 0:b - a],
                        mybir.ActivationFunctionType.Sqrt,
                        bias=bias_sb[ds][:, t:t + 1], scale=1.0)
                    dsc2 = dscr_pool.tile([TILE_P, PSUM_W], F16, tag="dsc2")
                    nc.vector.tensor_scalar(
                        dsc2[:, 0:b - a], dsc[:, 0:b - a], 1.0, None,
                        op0=mybir.AluOpType.mult, op1=mybir.AluOpType.add,
                        accum_out=acc_sb[:, col_:col_ + 1])
                else:
                    nc.scalar.activation(
                        dsc[:, 0:b - a], d2[:, 0:b - a],
                        mybir.ActivationFunctionType.Sqrt,
                        bias=bias_sb[ds][:, t:t + 1], scale=1.0,
                        accum_out=acc_sb[:, col_:col_ + 1])

            # class centers: onehot^T @ [x | 1]
            xna = xo_sb[ds][:, t * XO_W:t * XO_W + DDIM + 1]
            ohe = xo_sb[ds][:, t * XO_W + DDIM + 1:(t + 1) * XO_W]
            cps = cent_pool.tile([CLASSES, DDIM + 1], F32)
            nc.tensor.matmul(cps[:], ohe, xna, start=True, stop=True)
            w = DDIM + 1
            if t == 0:
                nc.vector.tensor_copy(cent_acc[:, ds * w:(ds + 1) * w], cps[:])
            else:
                nc.vector.tensor_add(
                    cent_acc[:, ds * w:(ds + 1) * w],
                    cent_acc[:, ds * w:(ds + 1) * w], cps[:])

        if acc_out is not None:
            # drain accum columns as they complete (finer for ds1 tail)
            if ds == 0:
                cols = [col_ for (t, w0, a, b, cls, col_) in layouts[ds]]
                c0, c1 = min(cols), max(cols) + 1
                nc.sync.dma_start(acc_out[:, c0:c1], acc_sb[:, c0:c1])
            else:
                by_t: dict = {}
                for (t, w0, a, b, cls, col_) in layouts[ds]:
                    by_t.setdefault(t, []).append(col_)
                for t, cols in by_t.items():
                    c0, c1 = min(cols), max(cols) + 1
                    nc.sync.dma_start(acc_out[:, c0:c1], acc_sb[:, c0:c1])


def _prepare_dataset(data, labels):
    """Sort by class; build device arrays. Returns a dict of host arrays."""
    data = np.ascontiguousarray(np.asarray(data, dtype=np.float32))
    labels = np.asarray(labels).astype(np.int64)
    order = np.argsort(labels, kind="stable")
    xs = data[order]                       # [N, D] f32 sorted
    ls = labels[order]                     # [N]
    counts = np.bincount(ls, minlength=CLASSES)
    bounds = tuple(int(v) for v in np.concatenate([[0], np.cumsum(counts)]))
    xh = xs.astype(np.float16)
    x2 = (xh.astype(np.float64) ** 2).sum(1)
    y2_hi = x2.astype(np.float16)
    y2_lo = (x2 - y2_hi.astype(np.float64)).astype(np.float16)
    rhs = np.ascontiguousarray(
        np.concatenate([xh.T, y2_hi[None], y2_lo[None]], axis=0))  # [66, N] f16
    lhsT = np.ascontiguousarray(np.concatenate(
        [(-2.0 * xh.astype(np.float32)).astype(np.float16).T,
         np.ones((2, N), np.float16)], axis=0))                    # [66, N] f16
    bias = (x2 + EPS).astype(np.float32).reshape(N, 1)             # [N,1] f32
    xo = np.ascontiguousarray(np.concatenate(
        [xs, np.ones((N, 1), np.float32),
         np.eye(CLASSES, dtype=np.float32)[ls]], axis=1))          # [N,69] f32
    # symmetric suffix-level row assignment: level t (cols [B_t, N)) takes
    # 1024 rows with b_class >= B_t; fill narrowest level first with the
    # highest classes (they fit the fewest levels... actually widest-eligible)
    Bs = _suffix_levels(bounds)
    unassigned = list(range(N - 1, -1, -1))  # highest class first (sorted order)
    level_rows = []
    for t in range(CLASSES - 1, -1, -1):
        take = []
        rest = []
        for r in unassigned:
            if len(take) < 1024 and int(bounds[ls[r]]) >= Bs[t]:
                take.append(r)
            else:
                rest.append(r)
        assert len(take) == 1024, f"level {t} underfilled: {len(take)}"
        take.sort()
        level_rows.append(take)
        unassigned = rest
    level_rows = level_rows[::-1]  # index by level t
    return dict(xs=xs, ls=ls, bounds=bounds, rhs=rhs, lhsT=lhsT,
                bias=bias, xo=xo, level_rows=np.array(level_rows))


def _run(inputs, trace=False):
    """Core implementation. Returns (result_scalar, BassKernelResults)."""
    assert np.asarray(inputs["source_data"]).shape == (N, DDIM), \
        "kernel compiled for source/target [4096, 64]"
    preps = [
        _prepare_dataset(inputs["source_data"], inputs["source_labels"]),
        _prepare_dataset(inputs["target_data"], inputs["target_labels"]),
    ]
    alpha = float(np.asarray(inputs["alpha"], dtype=np.float64))
    bounds_pair = (preps[0]["bounds"], preps[1]["bounds"])

    if bounds_pair not in _program_cache:
        _program_cache[bounds_pair] = _build_program(bounds_pair)
    nc, layouts = _program_cache[bounds_pair]

    in_maps = []
    perms = []  # [core][ds] -> row permutation (level-major, 512 rows)
    for core in range(NCORES):
        m = {}
        cp = []
        for ds, p in enumerate(preps):
            perm = np.concatenate(
                [p["level_rows"][t][core * TILE_P:(core + 1) * TILE_P]
                 for t in range(NTILES)])
            cp.append(perm)
            m[f"rhs{ds}"] = p["rhs"]
            m[f"lhs{ds}"] = np.ascontiguousarray(p["lhsT"][:, perm])
            m[f"bias{ds}"] = np.ascontiguousarray(
                p["bias"][perm].reshape(NTILES, TILE_P))
            m[f"xo{ds}"] = np.ascontiguousarray(
                p["xo"][perm].reshape(NTILES, TILE_P, -1))
        perms.append(cp)
        in_maps.append(m)

    res = run_bass_kernel_spmd(nc, in_maps, core_ids=list(range(NCORES)),
                               trace=False)

    # ---- host combination (symmetric coverage) ----
    # each computed (row c, col-class b) entry: b == c -> same-class pair,
    # counted once (incl. diagonal); b > c -> diff-class pair, represents
    # both orderings (weight 2); b < c -> duplicate coverage, discarded.
    diag = N * np.sqrt(EPS)
    total = 0.0
    cent_sum = np.zeros((CLASSES, 2 * (DDIM + 1)), np.float64)
    terms = []
    for ds, p in enumerate(preps):
        C = 0.0
        S2 = 0.0
        ls = p["ls"]
        for core in range(NCORES):
            acc = res.results[core]["acc"].astype(np.float64)  # [128, ACC_COLS]
            for (t, w0, a, b, cls, col_) in layouts[ds]:
                vec = acc[:, col_]
                rowcls = ls[perms[core][ds][t * TILE_P:(t + 1) * TILE_P]]
                C += vec[rowcls == cls].sum()
                S2 += vec[rowcls < cls].sum()
        C -= diag
        S = 2.0 * S2
        terms.append((0.5 * C - alpha * S) / N)
        total += terms[-1]
    for core in range(NCORES):
        cent_sum += res.results[core]["cent"].astype(np.float64)
    w = DDIM + 1
    cs = cent_sum[:, :w]
    ct = cent_sum[:, w:]
    src_centers = cs[:, :DDIM] / cs[:, DDIM:DDIM + 1]
    tgt_centers = ct[:, :DDIM] / ct[:, DDIM:DDIM + 1]
    diff = src_centers - tgt_centers
    total += float(np.sqrt((diff ** 2).sum(1)).mean())

    return np.array(total, dtype=np.float32), res


def kernel(**inputs):
    out, _ = _run(inputs, trace=False)
    return out


def run_traced(inputs):
    """For test harness: returns (output, BassKernelResults). NTFF tracing
    is unavailable on this axon client, so this is the untraced run."""
    return _run(inputs, trace=False)
